# revision 2
# baseline (speedup 1.0000x reference)
"""Trainium2 Bass kernel for nn_HHEdgeCons (moe_routing).

Strategy (8 NeuronCores, data-parallel over the node dim N=4096):
  - Each core gets 512 nodes -> 2048 rows r=(n,t) of length M=4096.
  - Device per r-block [128, 4096]:
      sel = max(lin,0)*mask        (one DVE scalar_tensor_tensor pass;
                                    valid because mask >= 0, and the
                                    accum_out gives the row L1 sums free)
      bn_stats/bn_aggr on sel      (row mean/var -> row sum-of-squares)
      PE: 128x128 fp32 transposes -> PSUM -> bf16 copy -> accumulate
          recon = sel @ fea        ([2048,4096]@[4096,64], bf16 weights)
  - Host: gathers sel shards (the big output), computes inp_projected
    from the tiny proj bank, and reduces the three norm sums (the
    "all-reduce") in float64 to form the scalar loss.

The kernel is HBM-bound: ~96 MB traffic/core ~= 270 us roofline.
"""

import numpy as np
import ml_dtypes

try:
    import concourse.bass as bass
except ImportError:  # pragma: no cover - fallback if not on default path
    import sys
    sys.path.insert(0, "/opt/trn_rl_repo")
    import concourse.bass as bass

import concourse.tile as tile
from concourse import bacc, mybir
from concourse.alu_op_type import AluOpType
from concourse.bass_utils import run_bass_kernel_spmd

N, T, F = 4096, 4, 64
NCORES = 8
NPC = N // NCORES            # nodes per core (512)
R = NPC * T                  # rows per core (2048)
M = N                        # contraction length (4096)
NB = R // 128                # r-blocks per core (16)
NMC = M // 128               # 128-wide m-chunks (32)
NG = NMC // 4                # transpose groups of 4 chunks (8)
RECONS_ERROR_LAMBDA = 0.1
L2_LAMBDA = 0.2
RECONS_LAMBDA = 0.01

_CACHE = {}


def _build():
    f32 = mybir.dt.float32
    bf16 = mybir.dt.bfloat16
    nc = bacc.Bacc("TRN2", target_bir_lowering=False, debug=False,
                   num_devices=NCORES)
    lin_d = nc.declare_dram_parameter("lin", [R, M], f32, isOutput=False)
    mask_d = nc.declare_dram_parameter("mask", [R, M], f32, isOutput=False)
    feab_d = nc.declare_dram_parameter("feab", [M, F], bf16, isOutput=False)
    ident_d = nc.declare_dram_parameter("ident", [128, 128], f32, isOutput=False)
    sel_d = nc.declare_dram_parameter("sel", [R, M], f32, isOutput=True)
    recon_d = nc.declare_dram_parameter("recon", [128, NB, F], f32, isOutput=True)
    rowl1_d = nc.declare_dram_parameter("rowl1", [128, NB], f32, isOutput=True)
    rowmv_d = nc.declare_dram_parameter("rowmv", [128, NB, 2], f32, isOutput=True)

    with tile.TileContext(nc) as tc:
        with (
            tc.tile_pool(name="singles", bufs=1) as singles,
            tc.tile_pool(name="lin_p", bufs=3) as lin_p,
            tc.tile_pool(name="mask_p", bufs=3) as mask_p,
            tc.tile_pool(name="sel_p", bufs=3) as sel_p,
            tc.tile_pool(name="stats_p", bufs=2) as stats_p,
            tc.tile_pool(name="selt_p", bufs=3) as selt_p,
            tc.tile_pool(name="psum_t", bufs=3, space="PSUM") as psum_t,
            tc.tile_pool(name="psum_r", bufs=2, space="PSUM") as psum_r,
        ):
            ident_sb = singles.tile([128, 128], f32)
            nc.sync.dma_start(ident_sb, ident_d[:, :])
            feab_sb = singles.tile([128, NMC, F], bf16)
            nc.sync.dma_start(feab_sb, feab_d.rearrange("(c p) f -> p c f", p=128))
            rowl1_sb = singles.tile([128, NB], f32)
            mv_sb = singles.tile([128, NB, 2], f32)
            recon_stage_sb = singles.tile([128, NB, F], f32)

            for rb in range(NB):
                rows = slice(rb * 128, (rb + 1) * 128)
                lin_t = lin_p.tile([128, M], f32)
                nc.sync.dma_start(lin_t, lin_d[rows, :])
                mask_t = mask_p.tile([128, M], f32)
                nc.sync.dma_start(mask_t, mask_d[rows, :])

                sel_t = sel_p.tile([128, M], f32)
                nc.vector.scalar_tensor_tensor(
                    out=sel_t, in0=lin_t, scalar=0.0, in1=mask_t,
                    op0=AluOpType.max, op1=AluOpType.mult,
                    accum_out=rowl1_sb[:, rb:rb + 1],
                )
                nc.scalar.dma_start(sel_d[rows, :], sel_t)

                stats = stats_p.tile([128, NG, 6], f32)
                for g in range(NG):
                    nc.vector.bn_stats(out=stats[:, g, :],
                                       in_=sel_t[:, g * 512:(g + 1) * 512])
                nc.vector.bn_aggr(out=mv_sb[:, rb, :], in_=stats)

                recon_ps = psum_r.tile([128, F], f32)
                for g in range(NG):
                    tp_ps = psum_t.tile([128, 512], f32)
                    for j in range(4):
                        mc = g * 4 + j
                        nc.tensor.transpose(
                            tp_ps[:, j * 128:(j + 1) * 128],
                            sel_t[:, mc * 128:(mc + 1) * 128],
                            ident_sb,
                        )
                    selt_sb = selt_p.tile([128, 512], bf16)
                    nc.any.tensor_copy(out=selt_sb, in_=tp_ps)
                    for j in range(4):
                        mc = g * 4 + j
                        nc.tensor.matmul(
                            recon_ps,
                            lhsT=selt_sb[:, j * 128:(j + 1) * 128],
                            rhs=feab_sb[:, mc, :],
                            start=(mc == 0), stop=(mc == NMC - 1),
                        )
                nc.any.tensor_copy(out=recon_stage_sb[:, rb, :], in_=recon_ps)

            nc.scalar.dma_start(recon_d[:, :, :], recon_stage_sb)
            nc.sync.dma_start(rowl1_d[:, :], rowl1_sb)
            nc.sync.dma_start(rowmv_d[:, :, :], mv_sb)

    nc.compile()
    return nc


def _get_nc():
    if "nc" not in _CACHE:
        _CACHE["nc"] = _build()
    return _CACHE["nc"]


def kernel(feature, mask, linear, proj, node_multi_mask, _trace=False):
    feature = np.asarray(feature)
    mask = np.asarray(mask)
    linear = np.asarray(linear)
    proj = np.asarray(proj)
    node_multi_mask = np.asarray(node_multi_mask)

    nc = _get_nc()

    lin_r = np.ascontiguousarray(linear.reshape(N, T, N))
    mask_r = np.ascontiguousarray(mask.reshape(N, T, N))
    feab = feature.astype(ml_dtypes.bfloat16)
    ident = np.eye(128, dtype=np.float32)

    in_maps = []
    for c in range(NCORES):
        nd = slice(c * NPC, (c + 1) * NPC)
        in_maps.append({
            "lin": np.ascontiguousarray(lin_r[nd]).reshape(R, M),
            "mask": np.ascontiguousarray(mask_r[nd]).reshape(R, M),
            "feab": feab,
            "ident": ident,
        })

    res = run_bass_kernel_spmd(nc, in_maps, list(range(NCORES)), trace=_trace)
    if _trace:
        _CACHE["last_results"] = res

    # ---- gather / unshard ----
    sel_full = np.empty((N, T, 1, N), dtype=np.float32)
    l1 = 0.0
    l2 = 0.0
    recon_all = np.empty((N * T, F), dtype=np.float32)
    for c in range(NCORES):
        r = res.results[c]
        sel_full[c * NPC:(c + 1) * NPC] = r["sel"].reshape(NPC, T, 1, N)
        l1 += float(r["rowl1"].sum(dtype=np.float64))
        mv = r["rowmv"].astype(np.float64)          # [128, NB, 2] (mean, var)
        sumsq = (mv[..., 1] + mv[..., 0] ** 2) * M  # [128, NB]
        l2 += float(np.sqrt(np.maximum(sumsq, 0.0)).sum())
        # recon[p, rb, :] -> local row index rb*128 + p, matching lin row order
        recon_all[c * R:(c + 1) * R] = r["recon"].transpose(1, 0, 2).reshape(R, F)

    # ---- inp_projected from the tiny proj bank (host; negligible work) ----
    allproj = np.einsum("ng,kfg->knf", feature, proj, optimize=True)  # [16,N,F]
    idx = node_multi_mask.astype(np.int64)[:, None] * T + np.arange(T)[None, :]
    inp_proj = allproj[idx, np.arange(N)[:, None], :]   # [N, T, F]
    diff = inp_proj.reshape(N * T, F).astype(np.float64) - recon_all.astype(np.float64)
    recon_error = float(np.sqrt((diff ** 2).sum(axis=1)).sum())

    recon_loss = RECONS_LAMBDA * recon_error + L2_LAMBDA * l2 + l1
    return sel_full, np.float32(RECONS_ERROR_LAMBDA * recon_loss)
